# revision 11
# baseline (speedup 1.0000x reference)
"""Trainium2 Bass kernel for nn_BaseSegmentTree (2-layer GNN over a fixed
segment-tree graph).  B=8 samples -> 8 NeuronCores, one sample per core.

Layout on device: feature-major [D=128 partitions, N=2048 nodes free].

Key ideas:
  * LN mean-centering is one PE matmul with C = I - J/128 (f32r full speed).
  * Variance lands compactly in a [16,128] PSUM tile via 16 accumulating
    selector matmuls; rsqrt is computed with an int-bit-hack seed + one
    Halley iteration on DVE (no ACT table switches), then broadcast back
    to [128,2048] with 16 selector matmuls.
  * gelu (exact) on ACT -- the only table set ever loaded.
  * The whole graph aggregation (descendant means for internal nodes +
    leaf attention windows, with 1/deg baked in) is a block-sparse PE
    matmul: A^T has 112 nonzero 128x128 blocks; stationary = 16 PE
    transposes of g, moving = packed bf16 A^T chunks.
  * w_nei/w_root matmuls accumulate into one PSUM tile; the residual add
    is a single scalar_tensor_tensor with b_nei folded in.
"""

import sys

sys.path.insert(0, "/opt/trn_rl_repo")

import numpy as np
import ml_dtypes
from contextlib import ExitStack

import concourse.bass as bass
import concourse.bacc as bacc
import concourse.tile as tile
import concourse.mybir as mybir
from concourse.bass_utils import run_bass_kernel_spmd

FP32 = mybir.dt.float32
BF16 = mybir.dt.bfloat16
F32R = mybir.dt.float32r
I32 = mybir.dt.int32
AF = mybir.ActivationFunctionType
OP = mybir.AluOpType

DEPTH = 10
LEAF = 2**DEPTH          # 1024
NODE_NUM = 2 * LEAF - 1  # 2047
NN = NODE_NUM + 1        # 2048 nodes incl. global node 0
D = 128
B = 8
EPS = 1e-5

_CACHE = {}


# --------------------------------------------------------------------------
# host-side constant construction
# --------------------------------------------------------------------------

def _pos_enc():
    """enc [NN, D] float32, with the global-node -1.0 folded into column 0."""
    def sinusoid(pos, d):
        half = d // 2
        inv = np.exp(-np.arange(half, dtype=np.float64) * (np.log(10000.0) / half))
        ang = pos[:, None] * inv[None, :]
        return np.stack([np.sin(ang), np.cos(ang)], -1).reshape(pos.shape[0], d)

    idx = np.arange(NN, dtype=np.float64)
    vpos = np.floor(np.log2(np.where(idx == 0, 0.5, idx)))
    hpos = idx - np.exp2(vpos)
    enc = np.concatenate([sinusoid(hpos, D // 2), sinusoid(vpos, D // 2)], -1)
    enc = enc.astype(np.float32)      # [NN, D]
    enc[0] += -1.0                    # node-0 feature is the constant -1
    return enc


def _build_A(edge_index):
    """Normalized adjacency A [NN, NN]: A[dst,src] = count/deg[dst] for one
    sample, built from (and validated against) the provided edge list."""
    src = np.asarray(edge_index[0], np.int64)
    dst = np.asarray(edge_index[1], np.int64)
    # edges are batch-offset by b*NN and identical across samples; use sample 0
    sample = (dst // NN) == 0
    s0, d0 = src[sample] % NN, dst[sample] % NN
    A = np.zeros((NN, NN), np.float64)
    np.add.at(A, (d0, s0), 1.0)
    deg = np.maximum(A.sum(1), 1.0)
    A /= deg[:, None]
    return A.astype(np.float32)


def _pack_blocks(A):
    """Pack nonzero 128x128 blocks of A^T into a contiguous bf16 operand.

    Returns (WT_packed [128, P] bf16, chunks). Each chunk is
    (src_block j, pack_col_off, width, dst_col_off, start, stop): a matmul
      agg_psum[:, dst:dst+w] (+)= g_T[:, 128j:128j+128].T @ WT[:, off:off+w]
    """
    AT = A.T  # [src, dst]
    nzb = np.zeros((16, 16), bool)
    for j in range(16):
        for b in range(16):
            nzb[j, b] = np.any(AT[128 * j:128 * (j + 1), 128 * b:128 * (b + 1)])
    first_j = {}
    last_j = {}
    for b in range(16):
        js = [j for j in range(16) if nzb[j, b]]
        assert js, f"dst block {b} has no sources"
        first_j[b] = js[0]
        last_j[b] = js[-1]

    # chunks must not cross PSUM bank boundaries (bank = 4 dst blocks =
    # 512 f32 cols): start=True lazily zeroes the WHOLE 2KB zero-region,
    # so accumulation groups are tracked per bank, not per 128-col block.
    raw = []   # (j, dst_block_start, nblocks) in emission order
    cols = []
    off = 0
    for j in range(16):
        bs = [b for b in range(16) if nzb[j, b]]
        runs = []
        for b in bs:
            if runs and runs[-1][-1] == b - 1:
                runs[-1].append(b)
            else:
                runs.append([b])
        for run in runs:
            # split at bank boundaries (block index multiple of 4)
            seg = []
            for b in run:
                if seg and (b // 4 != seg[0] // 4):
                    raw.append((j, seg[0], len(seg)))
                    seg = []
                seg.append(b)
            if seg:
                raw.append((j, seg[0], len(seg)))
    # split further so each chunk's dst blocks are uniformly fresh/written
    # at emission time (per-byte zero-consume must be uniform per matmul)
    written = set()
    raw2 = []
    for (j, b0, nb) in raw:
        seg = []
        for b in range(b0, b0 + nb):
            fresh = b not in written
            if seg and fresh != seg_fresh:
                raw2.append((j, seg[0], len(seg)))
                seg = []
            seg.append(b)
            seg_fresh = fresh
        if seg:
            raw2.append((j, seg[0], len(seg)))
        written.update(range(b0, b0 + nb))
    # per-bank first/last toucher in emission order -> start/stop flags
    bank_touch = {}
    for idx, (j, b0, nb) in enumerate(raw2):
        bank_touch.setdefault(b0 // 4, []).append(idx)
    chunks = []
    for idx, (j, b0, nb) in enumerate(raw2):
        bank = b0 // 4
        st = bank_touch[bank][0] == idx
        sp = bank_touch[bank][-1] == idx
        chunks.append((j, off, 128 * nb, 128 * b0, st, sp))
        cols.append(AT[128 * j:128 * (j + 1), 128 * b0:128 * (b0 + nb)])
        off += 128 * nb
    WT = np.concatenate(cols, axis=1).astype(ml_dtypes.bfloat16)
    return WT, chunks


# --------------------------------------------------------------------------
# device program
# --------------------------------------------------------------------------

def _build_program(pack_cols, chunks, n_layers, gamma_trivial, beta_trivial,
                   bnei_trivial):
    """Build + compile the Bacc program. Returns nc."""
    nc = bacc.Bacc("TRN2", target_bir_lowering=False, debug=False,
                   num_devices=B)

    elem_d = nc.dram_tensor("elem", [128, LEAF], FP32, kind="ExternalInput").ap()
    # cst32: enc [*,0:2048] | b_nei | gamma | beta
    c32_cols = NN + 3 * n_layers
    cst32_d = nc.dram_tensor("cst32", [128, c32_cols], FP32,
                             kind="ExternalInput").ap()
    cmat_d = nc.dram_tensor("cmat", [128, 128], F32R,
                            kind="ExternalInput").ap()
    # cstbf: WT [*,0:P] | identity | w_nei l0,l1 | w_root l0,l1
    #        | sel [2048] | onescol [256]
    cbf_cols = pack_cols + 128 + 4 * 128 + NN + 256
    cstbf_d = nc.dram_tensor("cstbf", [128, cbf_cols], BF16,
                             kind="ExternalInput").ap()
    out_d = nc.dram_tensor("out", [128, NN], FP32, kind="ExternalOutput").ap()

    MAGIC = 0x5F3759DF

    with tile.TileContext(nc) as tc, ExitStack() as ctx:
        cpool = ctx.enter_context(tc.tile_pool(name="const", bufs=1))
        wpool = ctx.enter_context(tc.tile_pool(name="work", bufs=1))
        spool = ctx.enter_context(tc.tile_pool(name="small", bufs=1))
        ppool = ctx.enter_context(tc.tile_pool(name="psum", bufs=1, space="PSUM"))
        tpool = ctx.enter_context(tc.tile_pool(name="tpsum", bufs=3, space="PSUM"))

        # ---- constants into SBUF ----
        e_sb = cpool.tile([128, LEAF], FP32, tag="e_sb")
        nc.sync.dma_start(out=e_sb[:], in_=elem_d[:])
        cst32 = cpool.tile([128, c32_cols], FP32, tag="cst32")
        nc.sync.dma_start(out=cst32[:], in_=cst32_d[:])
        cmat_sb = cpool.tile([128, 128], F32R, tag="cmat")
        nc.sync.dma_start(out=cmat_sb[:], in_=cmat_d[:])
        cstbf = cpool.tile([128, cbf_cols], BF16, tag="cstbf")
        # split the big constant DMA so it spreads across queues
        third = (cbf_cols // 3) & ~127
        nc.sync.dma_start(out=cstbf[:, :third], in_=cstbf_d[:, :third])
        nc.sync.dma_start(out=cstbf[:, third:2 * third],
                          in_=cstbf_d[:, third:2 * third])
        nc.sync.dma_start(out=cstbf[:, 2 * third:], in_=cstbf_d[:, 2 * third:])

        enc = cst32[:, 0:NN]
        Cmat = cmat_sb
        bnei_col = lambda l: cst32[:, NN + l:NN + l + 1]
        gam_col = lambda l: cst32[:, NN + n_layers + l:NN + n_layers + l + 1]
        bet_col = lambda l: cst32[:, NN + 2 * n_layers + l:NN + 2 * n_layers + l + 1]
        WT = cstbf[:, 0:pack_cols]
        ident = cstbf[:, pack_cols:pack_cols + 128]
        wnei = lambda l: cstbf[:, pack_cols + 128 + 128 * l:pack_cols + 128 + 128 * (l + 1)]
        wroot = lambda l: cstbf[:, pack_cols + 384 + 128 * l:pack_cols + 384 + 128 * (l + 1)]
        # sel [16, 16*128]: column block j has row j = 1.0 (bcast matmuls)
        sel = cstbf[0:16, pack_cols + 640:pack_cols + 640 + NN]
        # onescol [128, 16*16]: block c is [128,16] with column c = 1/128
        onescol = cstbf[:, pack_cols + 640 + NN:pack_cols + 640 + NN + 256]

        # force the gelu table set to load immediately (overlaps input DMA)
        dummy = spool.tile([128, 8], BF16, tag="dummy")
        nc.vector.memset(dummy[:], 0.0)
        nc.scalar.activation(dummy[:], dummy[:], AF.Gelu)

        # ---- tree compression -> x = node_feat + enc ----
        x_sb = wpool.tile([128, NN], F32R, tag="x")
        S = wpool.tile([128, LEAF], FP32, tag="S")  # raw subtree sums, cols 1..1023
        ev = e_sb.rearrange("p (n t) -> p n t", t=2)
        nc.vector.tensor_add(S[:, 512:1024], ev[:, :, 0], ev[:, :, 1])
        for v in range(8, -1, -1):
            lo, hi = 1 << v, 1 << (v + 1)
            sv = S[:, hi:2 * hi].rearrange("p (n t) -> p n t", t=2)
            nc.vector.tensor_add(S[:, lo:hi], sv[:, :, 0], sv[:, :, 1])
        # x internal levels: x_v = S_v * 2^(v-10) + enc_v
        for v in range(10):
            lo, hi = 1 << v, 1 << (v + 1)
            nc.vector.scalar_tensor_tensor(
                out=x_sb[:, lo:hi], in0=S[:, lo:hi], scalar=float(2.0 ** (v - 10)),
                in1=enc[:, lo:hi], op0=OP.mult, op1=OP.add)
        nc.vector.tensor_add(x_sb[:, LEAF:NN], e_sb[:], enc[:, LEAF:NN])
        nc.vector.tensor_copy(x_sb[:, 0:1], enc[:, 0:1])

        # ---- layers ----
        for l in range(n_layers):
            # LN centering: d = C @ x   (f32r, full speed at N=512)
            d_ps = ppool.tile([128, NN], FP32, tag="big")
            for nchunk in range(4):
                sl = slice(512 * nchunk, 512 * (nchunk + 1))
                nc.tensor.matmul(d_ps[:, sl], Cmat[:],
                                 x_sb[:, sl], start=True, stop=True)
            # square on ACT; bf16 copy of d on DVE
            sq_sb = wpool.tile([128, NN], BF16, tag="sq")
            nc.scalar.activation(sq_sb[:], d_ps[:], AF.Square)
            d_sb = wpool.tile([128, NN], BF16, tag="d")
            nc.vector.tensor_copy(d_sb[:], d_ps[:])

            # compact variance [16, 128]: 16 accumulating selector matmuls
            var_ps = ppool.tile([16, 128], FP32, tag="var")
            for c in range(16):
                nc.tensor.matmul(var_ps[:], onescol[:, 16 * c:16 * (c + 1)],
                                 sq_sb[:, 128 * c:128 * (c + 1)],
                                 start=(c == 0), stop=(c == 15))

            # rstd = rsqrt(var + eps): bit-hack seed + one Halley iteration
            v_sb = spool.tile([16, 128], FP32, tag="v")
            y_sb = spool.tile([16, 128], FP32, tag="y")
            w_sb = spool.tile([16, 128], FP32, tag="w")
            p_sb = spool.tile([16, 128], FP32, tag="p")
            s_sb = spool.tile([16, 128], FP32, tag="s")
            rstd_bf = spool.tile([16, 128], BF16, tag="rstd")
            nc.vector.tensor_scalar(out=v_sb[:], in0=var_ps[:], scalar1=EPS,
                                    scalar2=None, op0=OP.add)
            # y0 bits = MAGIC - (v_bits >> 1)  ==  (~(v_bits>>1)) + (MAGIC+1)
            nc.vector.tensor_scalar(out=w_sb.bitcast(I32)[:],
                                    in0=v_sb.bitcast(I32)[:],
                                    scalar1=1, scalar2=-1,
                                    op0=OP.logical_shift_right,
                                    op1=OP.bitwise_xor)
            nc.vector.tensor_scalar(out=y_sb.bitcast(I32)[:],
                                    in0=w_sb.bitcast(I32)[:],
                                    scalar1=MAGIC + 1, scalar2=None, op0=OP.add)
            # Halley: y1 = y0 * (0.375 p^2 - 1.25 p + 1.875), p = v*y0^2
            nc.vector.tensor_mul(w_sb[:], v_sb[:], y_sb[:])
            nc.vector.tensor_mul(p_sb[:], w_sb[:], y_sb[:])
            nc.vector.tensor_scalar(out=s_sb[:], in0=p_sb[:], scalar1=3.0,
                                    scalar2=-10.0, op0=OP.mult, op1=OP.add)
            nc.vector.tensor_mul(s_sb[:], s_sb[:], p_sb[:])
            nc.vector.tensor_scalar(out=s_sb[:], in0=s_sb[:], scalar1=0.125,
                                    scalar2=1.875, op0=OP.mult, op1=OP.add)
            nc.vector.tensor_mul(rstd_bf[:], y_sb[:], s_sb[:])

            # broadcast rstd back to [128, 2048] via selector matmuls
            r_ps = ppool.tile([128, NN], FP32, tag="big")
            for j in range(16):
                nc.tensor.matmul(r_ps[:, 128 * j:128 * (j + 1)],
                                 sel[:, 128 * j:128 * (j + 1)], rstd_bf[:],
                                 start=(j % 4 == 0), stop=(j % 4 == 3),
                                 skip_group_check=True)

            # h = d * rstd (* gamma + beta);   g = gelu(h)
            h_sb = wpool.tile([128, NN], BF16, tag="h")
            nc.vector.tensor_mul(h_sb[:], d_sb[:], r_ps[:])
            if not (gamma_trivial and beta_trivial):
                nc.vector.tensor_scalar(out=h_sb[:], in0=h_sb[:],
                                        scalar1=gam_col(l), scalar2=bet_col(l),
                                        op0=OP.mult, op1=OP.add)
            g_sb = wpool.tile([128, NN], BF16, tag="g")
            nc.scalar.activation(g_sb[:], h_sb[:], AF.Gelu)

            # transpose g -> g_T (node-major stationary for the agg matmuls)
            gT = wpool.tile([128, NN], BF16, tag="gT")
            for j in range(16):
                t_ps = tpool.tile([128, 128], BF16, tag="tp")
                nc.tensor.transpose(t_ps[:], g_sb[:, 128 * j:128 * (j + 1)],
                                    ident)
                if j % 2 == 0:
                    nc.scalar.copy(gT[:, 128 * j:128 * (j + 1)], t_ps[:])
                else:
                    nc.vector.tensor_copy(gT[:, 128 * j:128 * (j + 1)], t_ps[:])

            # block-sparse aggregation: agg = A @ g   (feature-major out)
            agg_ps = ppool.tile([128, NN], FP32, tag="big")
            for (j, off, width, dstoff, st, sp) in chunks:
                nc.tensor.matmul(agg_ps[:, dstoff:dstoff + width],
                                 gT[:, 128 * j:128 * (j + 1)],
                                 WT[:, off:off + width],
                                 start=st, stop=sp, skip_group_check=True)
            agg_sb = wpool.tile([128, NN], BF16, tag="agg")
            nc.vector.tensor_copy(agg_sb[:], agg_ps[:])

            # out = agg @ w_nei + g @ w_root (+ b_nei) + x
            o_ps = ppool.tile([128, NN], FP32, tag="big")
            for nchunk in range(4):
                sl = slice(512 * nchunk, 512 * (nchunk + 1))
                nc.tensor.matmul(o_ps[:, sl], wroot(l), g_sb[:, sl],
                                 start=True, stop=False)
                nc.tensor.matmul(o_ps[:, sl], wnei(l), agg_sb[:, sl],
                                 start=False, stop=True)
            if bnei_trivial:
                nc.vector.tensor_add(x_sb[:], o_ps[:], x_sb[:])
            else:
                nc.vector.scalar_tensor_tensor(
                    out=x_sb[:], in0=o_ps[:], scalar=bnei_col(l), in1=x_sb[:],
                    op0=OP.add, op1=OP.add)

        nc.sync.dma_start(out=out_d[:], in_=x_sb.bitcast(FP32)[:])

    nc.compile()
    return nc


# --------------------------------------------------------------------------
# public entry point
# --------------------------------------------------------------------------

def _get_compiled(inputs):
    key = "prog"
    if key in _CACHE:
        return _CACHE[key]

    ln_gamma = np.asarray(inputs["ln_gamma"], np.float32)
    ln_beta = np.asarray(inputs["ln_beta"], np.float32)
    w_nei = np.asarray(inputs["w_nei"], np.float32)
    b_nei = np.asarray(inputs["b_nei"], np.float32)
    w_root = np.asarray(inputs["w_root"], np.float32)
    edge_index = np.asarray(inputs["edge_index"])
    n_layers = ln_gamma.shape[0]

    A = _build_A(edge_index)
    WTpack, chunks = _pack_blocks(A)
    pack_cols = WTpack.shape[1]
    enc = _pos_enc()  # [NN, D]

    gamma_trivial = bool(np.all(ln_gamma == 1.0))
    beta_trivial = bool(np.all(ln_beta == 0.0))
    bnei_trivial = bool(np.all(b_nei == 0.0))

    # cst32 assembly
    c32_cols = NN + 3 * n_layers
    cst32 = np.zeros((128, c32_cols), np.float32)
    cst32[:, 0:NN] = enc.T
    cmat = (np.eye(128, dtype=np.float32)
            - np.full((128, 128), 1.0 / 128.0, np.float32))
    for l in range(n_layers):
        cst32[:, NN + l] = b_nei[l]
        cst32[:, NN + n_layers + l] = ln_gamma[l]
        cst32[:, NN + 2 * n_layers + l] = ln_beta[l]

    # cstbf assembly
    cbf_cols = pack_cols + 128 + 4 * 128 + NN + 256
    cstbf = np.zeros((128, cbf_cols), ml_dtypes.bfloat16)
    cstbf[:, 0:pack_cols] = WTpack
    cstbf[:, pack_cols:pack_cols + 128] = np.eye(128, dtype=np.float32)
    for l in range(n_layers):
        cstbf[:, pack_cols + 128 + 128 * l:pack_cols + 128 + 128 * (l + 1)] = \
            w_nei[l].astype(ml_dtypes.bfloat16)
        cstbf[:, pack_cols + 384 + 128 * l:pack_cols + 384 + 128 * (l + 1)] = \
            w_root[l].astype(ml_dtypes.bfloat16)
    for j in range(16):  # sel: row j = 1 in column block j
        cstbf[j, pack_cols + 640 + 128 * j:pack_cols + 640 + 128 * (j + 1)] = 1.0
    for c in range(16):  # onescol: block c has column c = 1/128
        cstbf[:, pack_cols + 640 + NN + 16 * c + c] = 1.0 / 128.0

    nc = _build_program(pack_cols, chunks, n_layers, gamma_trivial,
                        beta_trivial, bnei_trivial)
    _CACHE[key] = (nc, cst32, cstbf, cmat)
    return _CACHE[key]


def kernel(**inputs):
    elements = np.asarray(inputs["elements"], np.float32)  # [B, LEAF, D]
    nc, cst32, cstbf, cmat = _get_compiled(inputs)

    in_maps = []
    for i in range(B):
        in_maps.append({
            "elem": np.ascontiguousarray(elements[i].T),  # [128, 1024]
            "cst32": cst32,
            "cstbf": cstbf,
            "cmat": cmat,
        })
    res = run_bass_kernel_spmd(nc, in_maps, core_ids=list(range(B)))
    out = np.stack([res.results[i]["out"].T for i in range(B)])  # [B, NN, D]
    return out.astype(np.float32)


# revision 14
# speedup vs baseline: 1.3260x; 1.3260x over previous
"""Trainium2 Bass kernel for nn_BaseSegmentTree (2-layer GNN over a fixed
segment-tree graph).  B=8 samples -> 8 NeuronCores, one sample per core.

Layout on device: feature-major [D=128 partitions, N=2048 nodes free].

Key ideas:
  * LN mean-centering is one PE matmul with C = I - J/128 (f32r full speed).
  * Variance lands compactly in a [16,128] PSUM tile via 16 accumulating
    selector matmuls; rsqrt is computed with an int-bit-hack seed + one
    Halley iteration on DVE (no ACT table switches), then broadcast back
    to [128,2048] with 16 selector matmuls.
  * gelu (exact) on ACT -- the only table set ever loaded.
  * The whole graph aggregation (descendant means for internal nodes +
    leaf attention windows, with 1/deg baked in) is a block-sparse PE
    matmul: A^T has 112 nonzero 128x128 blocks; stationary = 16 PE
    transposes of g, moving = packed bf16 A^T chunks.
  * w_nei/w_root matmuls accumulate into one PSUM tile; the residual add
    is a single scalar_tensor_tensor with b_nei folded in.
"""

import sys

sys.path.insert(0, "/opt/trn_rl_repo")

import numpy as np
import ml_dtypes
from contextlib import ExitStack

import concourse.bass as bass
import concourse.bacc as bacc
import concourse.tile as tile
import concourse.mybir as mybir
from concourse.bass_utils import run_bass_kernel_spmd

FP32 = mybir.dt.float32
BF16 = mybir.dt.bfloat16
F32R = mybir.dt.float32r
I32 = mybir.dt.int32
AF = mybir.ActivationFunctionType
OP = mybir.AluOpType

DEPTH = 10
LEAF = 2**DEPTH          # 1024
NODE_NUM = 2 * LEAF - 1  # 2047
NN = NODE_NUM + 1        # 2048 nodes incl. global node 0
D = 128
B = 8
EPS = 1e-5

_CACHE = {}


# --------------------------------------------------------------------------
# host-side constant construction
# --------------------------------------------------------------------------

def _pos_enc():
    """enc [NN, D] float32, with the global-node -1.0 folded into column 0."""
    def sinusoid(pos, d):
        half = d // 2
        inv = np.exp(-np.arange(half, dtype=np.float64) * (np.log(10000.0) / half))
        ang = pos[:, None] * inv[None, :]
        return np.stack([np.sin(ang), np.cos(ang)], -1).reshape(pos.shape[0], d)

    idx = np.arange(NN, dtype=np.float64)
    vpos = np.floor(np.log2(np.where(idx == 0, 0.5, idx)))
    hpos = idx - np.exp2(vpos)
    enc = np.concatenate([sinusoid(hpos, D // 2), sinusoid(vpos, D // 2)], -1)
    enc = enc.astype(np.float32)      # [NN, D]
    enc[0] += -1.0                    # node-0 feature is the constant -1
    return enc


def _build_A(edge_index):
    """Normalized adjacency A [NN, NN]: A[dst,src] = count/deg[dst] for one
    sample, built from (and validated against) the provided edge list."""
    src = np.asarray(edge_index[0], np.int64)
    dst = np.asarray(edge_index[1], np.int64)
    # edges are batch-offset by b*NN and identical across samples; use sample 0
    sample = (dst // NN) == 0
    s0, d0 = src[sample] % NN, dst[sample] % NN
    A = np.zeros((NN, NN), np.float64)
    np.add.at(A, (d0, s0), 1.0)
    deg = np.maximum(A.sum(1), 1.0)
    A /= deg[:, None]
    return A.astype(np.float32)


def _pack_blocks(A):
    """Pack nonzero 128x128 blocks of A^T into a contiguous bf16 operand.

    Returns (WT_packed [128, P] bf16, chunks). Each chunk is
    (src_block j, pack_col_off, width, dst_col_off, start, stop): a matmul
      agg_psum[:, dst:dst+w] (+)= g_T[:, 128j:128j+128].T @ WT[:, off:off+w]
    """
    AT = A.T  # [src, dst]
    nzb = np.zeros((16, 16), bool)
    for j in range(16):
        for b in range(16):
            nzb[j, b] = np.any(AT[128 * j:128 * (j + 1), 128 * b:128 * (b + 1)])
    first_j = {}
    last_j = {}
    for b in range(16):
        js = [j for j in range(16) if nzb[j, b]]
        assert js, f"dst block {b} has no sources"
        first_j[b] = js[0]
        last_j[b] = js[-1]

    # chunks must not cross PSUM bank boundaries (bank = 4 dst blocks =
    # 512 f32 cols): start=True lazily zeroes the WHOLE 2KB zero-region,
    # so accumulation groups are tracked per bank, not per 128-col block.
    raw = []   # (j, dst_block_start, nblocks) in emission order
    cols = []
    off = 0
    for j in range(16):
        bs = [b for b in range(16) if nzb[j, b]]
        runs = []
        for b in bs:
            if runs and runs[-1][-1] == b - 1:
                runs[-1].append(b)
            else:
                runs.append([b])
        for run in runs:
            # split at bank boundaries (block index multiple of 4)
            seg = []
            for b in run:
                if seg and (b // 4 != seg[0] // 4):
                    raw.append((j, seg[0], len(seg)))
                    seg = []
                seg.append(b)
            if seg:
                raw.append((j, seg[0], len(seg)))
    # split further so each chunk's dst blocks are uniformly fresh/written
    # at emission time (per-byte zero-consume must be uniform per matmul)
    written = set()
    raw2 = []
    for (j, b0, nb) in raw:
        seg = []
        for b in range(b0, b0 + nb):
            fresh = b not in written
            if seg and fresh != seg_fresh:
                raw2.append((j, seg[0], len(seg)))
                seg = []
            seg.append(b)
            seg_fresh = fresh
        if seg:
            raw2.append((j, seg[0], len(seg)))
        written.update(range(b0, b0 + nb))
    # per-bank first/last toucher in emission order -> start/stop flags
    bank_touch = {}
    for idx, (j, b0, nb) in enumerate(raw2):
        bank_touch.setdefault(b0 // 4, []).append(idx)
    chunks = []
    for idx, (j, b0, nb) in enumerate(raw2):
        bank = b0 // 4
        st = bank_touch[bank][0] == idx
        sp = bank_touch[bank][-1] == idx
        chunks.append((j, off, 128 * nb, 128 * b0, st, sp))
        cols.append(AT[128 * j:128 * (j + 1), 128 * b0:128 * (b0 + nb)])
        off += 128 * nb
    WT = np.concatenate(cols, axis=1).astype(ml_dtypes.bfloat16)
    return WT, chunks


# --------------------------------------------------------------------------
# device program
# --------------------------------------------------------------------------

def _build_program(pack_cols, chunks, n_layers, gamma_trivial, beta_trivial,
                   bnei_trivial):
    """Build + compile the Bacc program. Returns nc."""
    nc = bacc.Bacc("TRN2", target_bir_lowering=False, debug=False,
                   num_devices=B)

    elem_d = nc.dram_tensor("elem", [128, LEAF], FP32, kind="ExternalInput").ap()
    # cst32: b_nei | gamma | beta   (tiny per-partition f32 columns)
    c32_cols = 3 * n_layers
    cst32_d = nc.dram_tensor("cst32", [128, c32_cols], FP32,
                             kind="ExternalInput").ap()
    cmat_d = nc.dram_tensor("cmat", [128, 128], F32R,
                            kind="ExternalInput").ap()
    # cstbf layout (DMA-priority order):
    #   enc [0:2048] | ident [2048:2176] | w_nei l0,l1 | w_root l0,l1
    #   | sel [2048] | onescol [256] | WT [pack_cols]
    HDR = NN + 128 + 4 * 128 + NN + 256
    cbf_cols = HDR + pack_cols
    cstbf_d = nc.dram_tensor("cstbf", [128, cbf_cols], BF16,
                             kind="ExternalInput").ap()
    out_d = nc.dram_tensor("out", [128, NN], FP32, kind="ExternalOutput").ap()

    MAGIC = 0x5F3759DF

    with tile.TileContext(nc) as tc, ExitStack() as ctx:
        cpool = ctx.enter_context(tc.tile_pool(name="const", bufs=1))
        wpool = ctx.enter_context(tc.tile_pool(name="work", bufs=1))
        spool = ctx.enter_context(tc.tile_pool(name="small", bufs=1))
        bpool = ctx.enter_context(tc.tile_pool(name="pbank", bufs=4, space="PSUM"))
        vpool = ctx.enter_context(tc.tile_pool(name="pvar", bufs=1, space="PSUM"))
        tpool = ctx.enter_context(tc.tile_pool(name="tpsum", bufs=3, space="PSUM"))

        # ---- input DMAs, spread across engines/queues ----
        e_sb = cpool.tile([128, LEAF], FP32, tag="e_sb")
        nc.sync.dma_start(out=e_sb[:], in_=elem_d[:])
        cst32 = cpool.tile([128, c32_cols], FP32, tag="cst32")
        cmat_sb = cpool.tile([128, 128], F32R, tag="cmat")
        cstbf = cpool.tile([128, cbf_cols], BF16, tag="cstbf")
        nc.scalar.dma_start(out=cstbf[:, :HDR], in_=cstbf_d[:, :HDR])
        nc.gpsimd.dma_start(out=cst32[:], in_=cst32_d[:])
        nc.gpsimd.dma_start(out=cmat_sb[:], in_=cmat_d[:])
        third = (pack_cols // 3) & ~127
        nc.gpsimd.dma_start(out=cstbf[:, HDR:HDR + third],
                            in_=cstbf_d[:, HDR:HDR + third])
        nc.scalar.dma_start(out=cstbf[:, HDR + third:HDR + 2 * third],
                            in_=cstbf_d[:, HDR + third:HDR + 2 * third])
        nc.sync.dma_start(out=cstbf[:, HDR + 2 * third:],
                          in_=cstbf_d[:, HDR + 2 * third:])

        enc = cstbf[:, 0:NN]
        ident = cstbf[:, NN:NN + 128]
        wnei = lambda l: cstbf[:, NN + 128 + 128 * l:NN + 128 + 128 * (l + 1)]
        wroot = lambda l: cstbf[:, NN + 384 + 128 * l:NN + 384 + 128 * (l + 1)]
        sel = cstbf[0:16, NN + 640:NN + 640 + NN]
        onescol = cstbf[:, 2 * NN + 640:2 * NN + 640 + 256]
        WT = cstbf[:, HDR:HDR + pack_cols]
        Cmat = cmat_sb
        bnei_col = lambda l: cst32[:, l:l + 1]
        gam_col = lambda l: cst32[:, n_layers + l:n_layers + l + 1]
        bet_col = lambda l: cst32[:, 2 * n_layers + l:2 * n_layers + l + 1]

        # force the gelu table set to load now (overlaps input DMA)
        dummy = spool.tile([128, 8], BF16, tag="dummy")
        nc.vector.memset(dummy[:], 0.0)
        nc.scalar.activation(dummy[:], dummy[:], AF.Gelu)

        # PE warm-up: ~4.3us of junk matmuls during the input DMA so the
        # HAM clock-gate is at 8/8 when real matmuls arrive
        wtile = spool.tile([128, 512], BF16, tag="wtile")
        nc.vector.memset(wtile[:], 0.0)
        warm_ps = vpool.tile([128, 512], FP32, tag="var")
        for _ in range(10):
            nc.tensor.matmul(warm_ps[:], wtile[:, 0:128], wtile[:],
                             start=True, stop=True)

        # ---- tree compression -> x = node_feat + enc ----
        x_sb = wpool.tile([128, NN], F32R, tag="x")
        S = wpool.tile([128, LEAF], FP32, tag="S")
        # leaves first: unlocks d-matmul chunks 2,3 early
        nc.vector.tensor_add(x_sb[:, LEAF:NN], e_sb[:], enc[:, LEAF:NN])
        ev = e_sb.rearrange("p (n t) -> p n t", t=2)
        nc.vector.tensor_add(S[:, 512:1024], ev[:, :, 0], ev[:, :, 1])
        for v in range(8, -1, -1):
            lo, hi = 1 << v, 1 << (v + 1)
            sv = S[:, hi:2 * hi].rearrange("p (n t) -> p n t", t=2)
            nc.vector.tensor_add(S[:, lo:hi], sv[:, :, 0], sv[:, :, 1])
        # x levels 9 first (chunk 1), then 0..8 + node0 (chunk 0)
        for v in [9] + list(range(9)):
            lo, hi = 1 << v, 1 << (v + 1)
            nc.vector.scalar_tensor_tensor(
                out=x_sb[:, lo:hi], in0=S[:, lo:hi], scalar=float(2.0 ** (v - 10)),
                in1=enc[:, lo:hi], op0=OP.mult, op1=OP.add)
        nc.vector.tensor_copy(x_sb[:, 0:1], enc[:, 0:1])

        # ---- layers (all phases chunked in 512-col banks) ----
        for l in range(n_layers):
            corder = [2, 3, 1, 0] if l == 0 else [0, 1, 2, 3]
            d_ps = {}
            sq_sb = wpool.tile([128, NN], BF16, tag="sq")
            d_sb = wpool.tile([128, NN], BF16, tag="d")
            var_ps = vpool.tile([16, 128], FP32, tag="var")
            for ci, c in enumerate(corder):
                sl = slice(512 * c, 512 * (c + 1))
                d_ps[c] = bpool.tile([128, 512], FP32, tag="bank", name=f"dps{c}")
                nc.tensor.matmul(d_ps[c][:], Cmat[:], x_sb[:, sl],
                                 start=True, stop=True)
                nc.scalar.activation(sq_sb[:, sl], d_ps[c][:], AF.Square)
                nc.vector.tensor_copy(d_sb[:, sl], d_ps[c][:])
                for k in range(4):
                    cc = 4 * c + k
                    nc.tensor.matmul(
                        var_ps[:], onescol[:, 16 * cc:16 * (cc + 1)],
                        sq_sb[:, 128 * cc:128 * (cc + 1)],
                        start=(ci == 0 and k == 0), stop=(ci == 3 and k == 3),
                        skip_group_check=True)

            # rstd = rsqrt(var+eps): bit-hack seed + one Halley iteration
            v_sb = spool.tile([16, 128], FP32, tag="v")
            y_sb = spool.tile([16, 128], FP32, tag="y")
            w_sb = spool.tile([16, 128], FP32, tag="w")
            p_sb = spool.tile([16, 128], FP32, tag="p")
            s_sb = spool.tile([16, 128], FP32, tag="s")
            rstd_bf = spool.tile([16, 128], BF16, tag="rstd")
            nc.vector.tensor_scalar(out=v_sb[:], in0=var_ps[:], scalar1=EPS,
                                    scalar2=None, op0=OP.add)
            nc.vector.tensor_scalar(out=w_sb.bitcast(I32)[:],
                                    in0=v_sb.bitcast(I32)[:],
                                    scalar1=1, scalar2=-1,
                                    op0=OP.logical_shift_right,
                                    op1=OP.bitwise_xor)
            nc.vector.tensor_scalar(out=y_sb.bitcast(I32)[:],
                                    in0=w_sb.bitcast(I32)[:],
                                    scalar1=MAGIC + 1, scalar2=None, op0=OP.add)
            nc.vector.tensor_mul(w_sb[:], v_sb[:], y_sb[:])
            nc.vector.tensor_mul(p_sb[:], w_sb[:], y_sb[:])
            nc.vector.tensor_scalar(out=s_sb[:], in0=p_sb[:], scalar1=3.0,
                                    scalar2=-10.0, op0=OP.mult, op1=OP.add)
            nc.vector.tensor_mul(s_sb[:], s_sb[:], p_sb[:])
            nc.vector.tensor_scalar(out=s_sb[:], in0=s_sb[:], scalar1=0.125,
                                    scalar2=1.875, op0=OP.mult, op1=OP.add)
            nc.vector.tensor_mul(rstd_bf[:], y_sb[:], s_sb[:])

            # rstd broadcast + h + gelu + transpose, pipelined per bank
            h_sb = wpool.tile([128, NN], BF16, tag="h")
            g_sb = wpool.tile([128, NN], BF16, tag="g")
            gT = wpool.tile([128, NN], BF16, tag="gT")
            for c in range(4):
                sl = slice(512 * c, 512 * (c + 1))
                r_ps = bpool.tile([128, 512], FP32, tag="bank")
                for q in range(4):
                    j = 4 * c + q
                    nc.tensor.matmul(r_ps[:, 128 * q:128 * (q + 1)],
                                     sel[:, 128 * j:128 * (j + 1)], rstd_bf[:],
                                     start=(q == 0), stop=(q == 3),
                                     skip_group_check=True)
                nc.vector.tensor_mul(h_sb[:, sl], d_sb[:, sl], r_ps[:])
                if not (gamma_trivial and beta_trivial):
                    nc.vector.tensor_scalar(out=h_sb[:, sl], in0=h_sb[:, sl],
                                            scalar1=gam_col(l), scalar2=bet_col(l),
                                            op0=OP.mult, op1=OP.add)
                nc.scalar.activation(g_sb[:, sl], h_sb[:, sl], AF.Gelu)
                for q in range(4):
                    j = 4 * c + q
                    t_ps = tpool.tile([128, 128], BF16, tag="tp")
                    nc.tensor.transpose(t_ps[:], g_sb[:, 128 * j:128 * (j + 1)],
                                        ident)
                    if q % 2 == 0:
                        nc.scalar.copy(gT[:, 128 * j:128 * (j + 1)], t_ps[:])
                    else:
                        nc.vector.tensor_copy(gT[:, 128 * j:128 * (j + 1)], t_ps[:])

            # block-sparse aggregation: agg = A @ g  (accumulated per bank)
            agg_ps = [bpool.tile([128, 512], FP32, tag="bank", name=f"aggps{i}")
                      for i in range(4)]
            for (j, off, width, dstoff, st, sp) in chunks:
                bank = dstoff // 512
                boff = dstoff - 512 * bank
                nc.tensor.matmul(agg_ps[bank][:, boff:boff + width],
                                 gT[:, 128 * j:128 * (j + 1)],
                                 WT[:, off:off + width],
                                 start=st, stop=sp, skip_group_check=True)

            # per bank: copy agg out (ACT), then w-matmuls reuse the bank,
            # then the residual add frees it
            agg_sb = wpool.tile([128, NN], BF16, tag="agg")
            for c in range(4):
                sl = slice(512 * c, 512 * (c + 1))
                nc.scalar.copy(agg_sb[:, sl], agg_ps[c][:])
                nc.tensor.matmul(agg_ps[c][:], wroot(l), g_sb[:, sl],
                                 start=True, stop=False)
                nc.tensor.matmul(agg_ps[c][:], wnei(l), agg_sb[:, sl],
                                 start=False, stop=True)
                if bnei_trivial:
                    nc.vector.tensor_add(x_sb[:, sl], agg_ps[c][:], x_sb[:, sl])
                else:
                    nc.vector.scalar_tensor_tensor(
                        out=x_sb[:, sl], in0=agg_ps[c][:], scalar=bnei_col(l),
                        in1=x_sb[:, sl], op0=OP.add, op1=OP.add)
                if l == n_layers - 1:
                    eng = [nc.sync, nc.gpsimd, nc.scalar, nc.sync][c]
                    eng.dma_start(out=out_d[:, sl], in_=x_sb.bitcast(FP32)[:, sl])

    nc.compile()
    return nc


# --------------------------------------------------------------------------
# public entry point
# --------------------------------------------------------------------------

def _get_compiled(inputs):
    key = "prog"
    if key in _CACHE:
        return _CACHE[key]

    ln_gamma = np.asarray(inputs["ln_gamma"], np.float32)
    ln_beta = np.asarray(inputs["ln_beta"], np.float32)
    w_nei = np.asarray(inputs["w_nei"], np.float32)
    b_nei = np.asarray(inputs["b_nei"], np.float32)
    w_root = np.asarray(inputs["w_root"], np.float32)
    edge_index = np.asarray(inputs["edge_index"])
    n_layers = ln_gamma.shape[0]

    A = _build_A(edge_index)
    WTpack, chunks = _pack_blocks(A)
    pack_cols = WTpack.shape[1]
    enc = _pos_enc()  # [NN, D]

    gamma_trivial = bool(np.all(ln_gamma == 1.0))
    beta_trivial = bool(np.all(ln_beta == 0.0))
    bnei_trivial = bool(np.all(b_nei == 0.0))

    # cst32 assembly (b_nei | gamma | beta)
    c32_cols = 3 * n_layers
    cst32 = np.zeros((128, c32_cols), np.float32)
    cmat = (np.eye(128, dtype=np.float32)
            - np.full((128, 128), 1.0 / 128.0, np.float32))
    for l in range(n_layers):
        cst32[:, l] = b_nei[l]
        cst32[:, n_layers + l] = ln_gamma[l]
        cst32[:, 2 * n_layers + l] = ln_beta[l]

    # cstbf assembly: enc | ident | w_nei | w_root | sel | onescol | WT
    HDR = NN + 128 + 4 * 128 + NN + 256
    cbf_cols = HDR + pack_cols
    cstbf = np.zeros((128, cbf_cols), ml_dtypes.bfloat16)
    cstbf[:, 0:NN] = enc.T
    cstbf[:, NN:NN + 128] = np.eye(128, dtype=np.float32)
    for l in range(n_layers):
        cstbf[:, NN + 128 + 128 * l:NN + 128 + 128 * (l + 1)] = \
            w_nei[l].astype(ml_dtypes.bfloat16)
        cstbf[:, NN + 384 + 128 * l:NN + 384 + 128 * (l + 1)] = \
            w_root[l].astype(ml_dtypes.bfloat16)
    for j in range(16):  # sel: row j = 1 in column block j
        cstbf[j, NN + 640 + 128 * j:NN + 640 + 128 * (j + 1)] = 1.0
    for c in range(16):  # onescol: block c has column c = 1/128
        cstbf[:, 2 * NN + 640 + 16 * c + c] = 1.0 / 128.0
    cstbf[:, HDR:HDR + pack_cols] = WTpack

    nc = _build_program(pack_cols, chunks, n_layers, gamma_trivial,
                        beta_trivial, bnei_trivial)
    _CACHE[key] = (nc, cst32, cstbf, cmat)
    return _CACHE[key]


def kernel(**inputs):
    elements = np.asarray(inputs["elements"], np.float32)  # [B, LEAF, D]
    nc, cst32, cstbf, cmat = _get_compiled(inputs)

    in_maps = []
    for i in range(B):
        in_maps.append({
            "elem": np.ascontiguousarray(elements[i].T),  # [128, 1024]
            "cst32": cst32,
            "cstbf": cstbf,
            "cmat": cmat,
        })
    res = run_bass_kernel_spmd(nc, in_maps, core_ids=list(range(B)))
    out = np.stack([res.results[i]["out"].T for i in range(B)])  # [B, NN, D]
    return out.astype(np.float32)
